# revision 1
# baseline (speedup 1.0000x reference)
"""Trainium2 Bass kernel: 16-head attention with LoRA (B=2, N=2048, C=1024).

Sharding: batch x sequence rows across 8 cores (core c: batch c//4, rows
(c%4)*512). Heads stay whole per core; K/V are all-gathered over the 4-core
batch group. Everything on device is computed transposed (feature dim on
partitions), so no on-device transposes are needed; the host transposes the
per-core [1024, 512] output slabs back.
"""

import os
from contextlib import ExitStack

import numpy as np
import ml_dtypes

import concourse.bass as bass
import concourse.mybir as mybir
import concourse.tile as tile
from concourse.bass_utils import run_bass_kernel_spmd

B, N, C, H, D = 2, 2048, 1024, 16, 64
R = 512          # query rows per core
KT = N // 128    # 16 seq tiles of 128
BF = mybir.dt.bfloat16
F32 = mybir.dt.float32
GROUPS = [[0, 1, 2, 3], [4, 5, 6, 7]]


def build():
    nc = bass.Bass()
    xT = nc.declare_dram_parameter("xT", [C, R], BF, isOutput=False)
    wqkT = nc.declare_dram_parameter("wqkT", [C, 2 * C], BF, isOutput=False)
    wvT = nc.declare_dram_parameter("wvT", [C, C], BF, isOutput=False)
    a1T = nc.declare_dram_parameter("a1T", [C, 8], BF, isOutput=False)
    b1qkT = nc.declare_dram_parameter("b1qkT", [8, 2 * C], BF, isOutput=False)
    b1vT = nc.declare_dram_parameter("b1vT", [8, C], BF, isOutput=False)
    projT = nc.declare_dram_parameter("projT", [C, C], BF, isOutput=False)
    a2T = nc.declare_dram_parameter("a2T", [C, 8], BF, isOutput=False)
    b2aug = nc.declare_dram_parameter("b2aug", [9, C], BF, isOutput=False)
    outT = nc.declare_dram_parameter("outT", [C, R], F32, isOutput=True)

    with tile.TileContext(nc) as tc, ExitStack() as ctx:
        dram = ctx.enter_context(tc.tile_pool(name="dram", bufs=1, space="DRAM"))
        k_bounce = dram.tile([C, R], BF)
        v_bounce = dram.tile([R, C], BF)
        k_gath = dram.tile([4 * C, R], BF)
        v_gath = dram.tile([N, C], BF)

        cst = ctx.enter_context(tc.tile_pool(name="cst", bufs=1))
        xT_s = cst.tile([128, 8, R], BF)
        nc.sync.dma_start(out=xT_s, in_=xT[:, :].rearrange("(kt p) r -> p kt r", p=128))
        a1T_s = cst.tile([128, 8, 8], BF)
        nc.sync.dma_start(out=a1T_s, in_=a1T[:, :].rearrange("(kt p) e -> p kt e", p=128))
        wqkT_s = cst.tile([128, 8, 2 * C], BF)
        nc.sync.dma_start(out=wqkT_s, in_=wqkT[:, :].rearrange("(kt p) c -> p kt c", p=128))
        wvT_s = cst.tile([128, 8, C], BF)
        nc.sync.dma_start(out=wvT_s, in_=wvT[:, :].rearrange("(kt p) c -> p kt c", p=128))
        b1qkT_s = cst.tile([8, 2 * C], BF)
        nc.sync.dma_start(out=b1qkT_s, in_=b1qkT[:, :])
        b1vT_s = cst.tile([8, C], BF)
        nc.sync.dma_start(out=b1vT_s, in_=b1vT[:, :])
        projT_s = cst.tile([128, 8, C], BF)
        nc.sync.dma_start(out=projT_s, in_=projT[:, :].rearrange("(kt p) c -> p kt c", p=128))
        a2T_s = cst.tile([128, 8, 8], BF)
        nc.sync.dma_start(out=a2T_s, in_=a2T[:, :].rearrange("(kt p) e -> p kt e", p=128))
        b2aug_s = cst.tile([9, C], BF)
        nc.sync.dma_start(out=b2aug_s, in_=b2aug[:, :])

        xaT_s = cst.tile([8, R], BF)
        qkT_s = cst.tile([128, 16, R], BF)
        v_ls = cst.tile([128, 4, C], BF)
        att_s = cst.tile([128, 8, R], BF)

        # ---- phase 1: lora1 intermediate xaT = A1 @ x.T  [8, R]
        with tc.tile_pool(name="psA", bufs=1, space="PSUM") as psA:
            xa_ps = psA.tile([8, R], F32, tag="sm")
            for kt in range(8):
                nc.tensor.matmul(xa_ps, a1T_s[:, kt, :], xT_s[:, kt, :],
                                 start=(kt == 0), stop=(kt == 7))
            nc.vector.tensor_copy(xaT_s, xa_ps)

            # ---- phase 2: qkT = (qkv_w[:2C] @ x.T + lora), k-part first
            for ct in list(range(8, 16)) + list(range(8)):
                qk_ps = psA.tile([128, R], F32, tag="mm", bufs=2)
                for kt in range(8):
                    nc.tensor.matmul(qk_ps, wqkT_s[:, kt, ct * 128:(ct + 1) * 128],
                                     xT_s[:, kt, :], start=(kt == 0), stop=False)
                nc.tensor.matmul(qk_ps, b1qkT_s[:, ct * 128:(ct + 1) * 128], xaT_s,
                                 start=False, stop=True)
                nc.vector.tensor_copy(qkT_s[:, ct, :], qk_ps)
                if ct == 15:
                    nc.gpsimd.dma_start(
                        out=k_bounce[:, :].rearrange("(ct p) r -> p ct r", p=128),
                        in_=qkT_s[:, 8:16, :])
                    nc.gpsimd.collective_compute(
                        "AllGather", mybir.AluOpType.bypass,
                        ins=[k_bounce.opt()], outs=[k_gath.opt()],
                        replica_groups=GROUPS)

            # ---- phase 3: v natural = x @ wv.T + lora   [R, C]
            for rt in range(4):
                for vct in range(2):
                    v_ps = psA.tile([128, 512], F32, tag="mm", bufs=2)
                    for kt in range(8):
                        nc.tensor.matmul(v_ps, xT_s[:, kt, rt * 128:(rt + 1) * 128],
                                         wvT_s[:, kt, vct * 512:(vct + 1) * 512],
                                         start=(kt == 0), stop=False)
                    nc.tensor.matmul(v_ps, xaT_s[:, rt * 128:(rt + 1) * 128],
                                     b1vT_s[:, vct * 512:(vct + 1) * 512],
                                     start=False, stop=True)
                    nc.vector.tensor_copy(v_ls[:, rt, vct * 512:(vct + 1) * 512], v_ps)
            nc.gpsimd.dma_start(
                out=v_bounce[:, :].rearrange("(rt p) c -> p rt c", p=128),
                in_=v_ls)
            nc.gpsimd.collective_compute(
                "AllGather", mybir.AluOpType.bypass,
                ins=[v_bounce.opt()], outs=[v_gath.opt()],
                replica_groups=GROUPS)

        v_gr = v_gath[:, :].rearrange("(kt p) c -> p kt c", p=128)

        # ---- phase 4+5 pools open together (no sem-frontier collapse at
        # a PSUM pool boundary; walrus caps sync waits per instruction)
        with tc.tile_pool(name="psB", bufs=1, space="PSUM") as psB, \
             tc.tile_pool(name="psC", bufs=1, space="PSUM") as psC, \
             tc.tile_pool(name="atn", bufs=1) as atn, \
             tc.tile_pool(name="prj", bufs=1) as prj:
            oa_aug = prj.tile([9, R], BF)
            nc.vector.memset(oa_aug, 1.0)
            for kp in range(8):
                kT_p = atn.tile([128, N], BF, tag="ktp", bufs=2)
                kg = k_gath[:, :]
                nc.sync.dma_start(
                    out=kT_p[:, :].rearrange("p (rk r) -> p rk r", rk=4),
                    in_=bass.AP(tensor=kg.tensor,
                                offset=kg.offset + kp * 128 * R,
                                ap=[[R, 128], [C * R, 4], [1, R]]))
                vs = []
                exps = []
                att_os = []
                for j in range(2):
                    h = 2 * kp + j
                    v_sj = atn.tile([128, KT, 65], BF, tag="vs", bufs=4)
                    nc.vector.memset(v_sj, 1.0)
                    nc.gpsimd.dma_start(out=v_sj[:, :, 0:64],
                                        in_=v_gr[:, :, h * 64:(h + 1) * 64])
                    vs.append(v_sj)
                    exps.append(atn.tile([128, KT, R], BF, tag="exps", bufs=2,
                                         name=f"exp_{kp}_{j}"))
                    att_os.append(psB.tile([65, R], F32, tag="ao", bufs=2,
                                           name=f"ao_{kp}_{j}"))
                # scores + exp, heads interleaved so K=64 row-groups pack
                for g in range(8):
                    sp = [psB.tile([128, 2, R], F32, tag="sc", bufs=2,
                                   name=f"sc_{kp}_{g}_{j}") for j in range(2)]
                    for jj in range(2):
                        kt = 2 * g + jj
                        for j in range(2):
                            nc.tensor.matmul(
                                sp[j][:, jj, :],
                                kT_p[j * 64:(j + 1) * 64, kt * 128:(kt + 1) * 128],
                                qkT_s[j * 64:(j + 1) * 64, kp, :],
                                start=True, stop=True)
                    for j in range(2):
                        nc.scalar.activation(exps[j][:, 2 * g:2 * g + 2, :], sp[j],
                                             mybir.ActivationFunctionType.Exp,
                                             scale=0.125)
                # attn @ V (transposed), with ones column giving the softmax denom
                for j in range(2):
                    for kt in range(KT):
                        nc.tensor.matmul(att_os[j], vs[j][:, kt, :], exps[j][:, kt, :],
                                         start=(kt == 0), stop=(kt == KT - 1))
                for j in range(2):
                    rr = atn.tile([65, R], F32, tag="rr", bufs=2)
                    nc.vector.reciprocal(rr[64:65, :], att_os[j][64:65, :])
                    rr_d = dram.tile([1, R], F32, tag="rrd", bufs=4,
                                     name=f"rrd_{kp}_{j}")
                    nc.gpsimd.dma_start(out=rr_d, in_=rr[64:65, :])
                    rd = rr_d[:, :]
                    rb = atn.tile([64, R], F32, tag="rb", bufs=2)
                    nc.sync.dma_start(out=rb, in_=bass.AP(
                        tensor=rd.tensor, offset=rd.offset,
                        ap=[[0, 64]] + [list(d) for d in rd.ap[1:]]))
                    if j == 0:
                        nc.vector.tensor_mul(att_s[0:64, kp, :], att_os[j][0:64, :], rb)
                    else:
                        tmp = atn.tile([64, R], BF, tag="atmp", bufs=2)
                        nc.vector.tensor_mul(tmp, att_os[j][0:64, :], rb)
                        nc.gpsimd.dma_start(out=att_s[64:128, kp, :], in_=tmp)

            # ---- phase 5: output projection with lora2 + bias
            oa_ps = psC.tile([8, R], F32, tag="sm")
            for kp in range(8):
                nc.tensor.matmul(oa_ps, a2T_s[:, kp, :], att_s[:, kp, :],
                                 start=(kp == 0), stop=(kp == 7))
            nc.vector.tensor_copy(oa_aug[0:8, :], oa_ps)
            for ct in range(8):
                f_ps = psC.tile([128, R], F32, tag="fm", bufs=1)
                for kp in range(8):
                    nc.tensor.matmul(f_ps, projT_s[:, kp, ct * 128:(ct + 1) * 128],
                                     att_s[:, kp, :], start=(kp == 0), stop=False)
                nc.tensor.matmul(f_ps, b2aug_s[:, ct * 128:(ct + 1) * 128], oa_aug,
                                 start=False, stop=True)
                f_s = prj.tile([128, R], F32, tag="fs", bufs=2)
                nc.vector.tensor_copy(f_s, f_ps)
                nc.gpsimd.dma_start(out=outT[ct * 128:(ct + 1) * 128, :], in_=f_s)
    _split_multi_waits(nc)
    return nc


def _split_multi_waits(nc):
    """This container's walrus supports one sync-wait per instruction; move
    extra waits onto preceding same-engine NoOps."""
    n_new = 0
    for bb in nc.m.functions[0].blocks:
        new = []
        for ins in bb.instructions:
            si = getattr(ins, "sync_info", None)
            ow = list(si.on_wait) if si is not None and si.on_wait else []
            if len(ow) > 1:
                for w in ow[:-1]:
                    n_new += 1
                    nop = mybir.InstNoOp(
                        name=f"{ins.name}_sw{n_new}",
                        engine=ins.engine,
                        sync_info=mybir.SyncInfo(on_wait=[w], on_update=[]),
                    )
                    new.append(nop)
                ins.sync_info = mybir.SyncInfo(
                    on_wait=[ow[-1]],
                    on_update=list(si.on_update) if si.on_update else [],
                )
            new.append(ins)
        bb.instructions = new


_NC = None
_LAST = None


def _ensure_ntff_hook():
    """The agent image's antenv lacks axon_hooks; shim it and register the
    ctypes NTFF profiler from trn_boot so trace=True yields exec_time_ns."""
    import sys
    import types
    try:
        import antenv.axon_hooks  # noqa: F401
        return
    except ImportError:
        pass
    mod = types.ModuleType("antenv.axon_hooks")
    holder = [None]
    mod.set_axon_ntff_profile_hook = lambda h: holder.__setitem__(0, h)
    mod.get_axon_ntff_profile_hook = lambda: holder[0]
    sys.modules["antenv.axon_hooks"] = mod
    import antenv
    antenv.axon_hooks = mod
    try:
        sys.path.insert(0, "/root/.axon_site")
        from trn_agent_boot.trn_boot import _ntff_profile_via_ctypes
        mod.set_axon_ntff_profile_hook(
            _ntff_profile_via_ctypes("/opt/axon/libaxon_pjrt.so"))
    except Exception:
        pass


def kernel(**inputs):
    global _NC, _LAST
    bf = ml_dtypes.bfloat16
    x = np.asarray(inputs["x"], np.float32)
    qkv_w = np.asarray(inputs["qkv_w"], np.float32)
    proj_w = np.asarray(inputs["proj_w"], np.float32)
    proj_b = np.asarray(inputs["proj_b"], np.float32)
    a1 = np.asarray(inputs["lora_w1_l1"], np.float32)
    b1 = np.asarray(inputs["lora_w1_l2"], np.float32)
    a2 = np.asarray(inputs["lora_w2_l1"], np.float32)
    b2 = np.asarray(inputs["lora_w2_l2"], np.float32)

    shared = {
        "wqkT": np.ascontiguousarray(qkv_w[:2 * C].T).astype(bf),
        "wvT": np.ascontiguousarray(qkv_w[2 * C:].T).astype(bf),
        "a1T": np.ascontiguousarray(a1.T).astype(bf),
        "b1qkT": np.ascontiguousarray(b1[:2 * C].T * 2.0).astype(bf),
        "b1vT": np.ascontiguousarray(b1[2 * C:].T * 2.0).astype(bf),
        "projT": np.ascontiguousarray(proj_w.T).astype(bf),
        "a2T": np.ascontiguousarray(a2.T).astype(bf),
        "b2aug": np.ascontiguousarray(
            np.vstack([b2.T * 2.0, proj_b[None, :]])).astype(bf),
    }
    in_maps = []
    for c in range(8):
        g, r = divmod(c, 4)
        m = dict(shared)
        m["xT"] = np.ascontiguousarray(x[g, r * R:(r + 1) * R, :].T).astype(bf)
        in_maps.append(m)

    if _NC is None:
        _NC = build()
    trace = os.environ.get("ATT_TRACE", "0") == "1"
    if trace:
        _ensure_ntff_hook()
    _LAST = run_bass_kernel_spmd(_NC, in_maps, core_ids=list(range(8)),
                                 trace=trace)
    out = np.empty((B, N, C), np.float32)
    for c in range(8):
        g, r = divmod(c, 4)
        out[g, r * R:(r + 1) * R, :] = np.asarray(
            _LAST.results[c]["outT"], np.float32).T
    return out



# revision 4
# speedup vs baseline: 1.2808x; 1.2808x over previous
"""Trainium2 Bass kernel: 16-head attention with LoRA (B=2, N=2048, C=1024).

Sharding: batch x sequence rows across 8 cores (core c: batch c//4, rows
(c%4)*512). Heads stay whole per core; K/V are all-gathered over the 4-core
batch group in chunks interleaved with compute. LoRA is folded into the
weights on the host (W_eff = W + 2*B@A), softmax normalization is deferred
and batched. Everything on device is computed transposed (feature dim on
partitions); the host transposes the per-core [1024, 512] output slabs back.
"""

import os
from contextlib import ExitStack

import numpy as np
import ml_dtypes

import concourse.bass as bass
import concourse.mybir as mybir
import concourse.tile as tile
from concourse.bass_utils import run_bass_kernel_spmd

B, N, C, H, D = 2, 2048, 1024, 16, 64
R = 512          # query rows per core
KT = N // 128    # 16 seq tiles of 128
BF = mybir.dt.bfloat16
F32 = mybir.dt.float32
GROUPS = [[0, 1, 2, 3], [4, 5, 6, 7]]


def _ap(src, dims):
    """Rebuild an AP keeping its partition dim but with custom free dims."""
    return bass.AP(tensor=src.tensor, offset=src.offset,
                   ap=[list(src.ap[0])] + [list(d) for d in dims])


def build():
    nc = bass.Bass()
    xT = nc.declare_dram_parameter("xT", [C, R], BF, isOutput=False)
    wkT = nc.declare_dram_parameter("wkT", [C, C], BF, isOutput=False)
    wqT = nc.declare_dram_parameter("wqT", [C, C], BF, isOutput=False)
    wvT = nc.declare_dram_parameter("wvT", [C, C], BF, isOutput=False)
    projT = nc.declare_dram_parameter("projT", [C, C], BF, isOutput=False)
    biasT = nc.declare_dram_parameter("biasT", [1, C], BF, isOutput=False)
    outT = nc.declare_dram_parameter("outT", [C, R], F32, isOutput=True)

    with tile.TileContext(nc) as tc, ExitStack() as ctx:
        dram = ctx.enter_context(tc.tile_pool(name="dram", bufs=1, space="DRAM"))
        warm_in = dram.tile([1, 128], BF)
        warm_out = dram.tile([4, 128], BF)
        kA_b = dram.tile([4 * 128, R], BF)
        kB_b = dram.tile([4 * 128, R], BF)
        kA_g = dram.tile([4 * 4 * 128, R], BF)
        kB_g = dram.tile([4 * 4 * 128, R], BF)
        vA_b = dram.tile([R, 520], BF)
        vB_b = dram.tile([R, 520], BF)
        vA_g = dram.tile([N, 520], BF)
        vB_g = dram.tile([N, 520], BF)
        den_d = dram.tile([16, R], F32)
        den_rd = dram.tile([16, R], BF)

        cst = ctx.enter_context(tc.tile_pool(name="cst", bufs=1))
        xT_s = cst.tile([128, 8, R], BF)
        nc.sync.dma_start(out=xT_s, in_=xT[:, :].rearrange("(kt p) r -> p kt r", p=128))
        wkT_s = cst.tile([128, 8, C], BF)
        nc.sync.dma_start(out=wkT_s, in_=wkT[:, :].rearrange("(kt p) c -> p kt c", p=128))
        wvT_s = cst.tile([128, 8, C], BF)
        nc.sync.dma_start(out=wvT_s, in_=wvT[:, :].rearrange("(kt p) c -> p kt c", p=128))
        wqT_s = cst.tile([128, 8, C], BF)
        nc.sync.dma_start(out=wqT_s, in_=wqT[:, :].rearrange("(kt p) c -> p kt c", p=128))
        projT_s = cst.tile([128, 8, C], BF)
        nc.sync.dma_start(out=projT_s, in_=projT[:, :].rearrange("(kt p) c -> p kt c", p=128))
        biasT_s = cst.tile([1, C], BF)
        nc.sync.dma_start(out=biasT_s, in_=biasT[:, :])

        ones_s = cst.tile([1, R], BF)
        nc.vector.memset(ones_s, 1.0)
        kT_ls = cst.tile([128, 8, R], BF)
        qT_s = cst.tile([128, 8, R], BF)
        v_ls = cst.tile([128, 4, 1040], BF)
        nc.vector.memset(v_ls, 1.0)
        vA_s = cst.tile([128, KT, 520], BF)
        vB_s = cst.tile([128, KT, 520], BF)
        att_un = cst.tile([128, 8, R], F32)
        att_s = cst.tile([128, 8, R], BF)
        rb_s = cst.tile([128, 8, R], BF)
        den_l = cst.tile([16, R], F32)
        den_r = cst.tile([16, R], BF)

        # warm-up collective at t~0: absorbs the ncfw barrier/setup latency
        warm_s = cst.tile([1, 128], BF)
        nc.vector.memset(warm_s, 1.0)
        nc.gpsimd.dma_start(out=warm_in, in_=warm_s)
        nc.gpsimd.collective_compute(
            "AllGather", mybir.AluOpType.bypass,
            ins=[warm_in.opt()], outs=[warm_out.opt()],
            replica_groups=GROUPS)

        atn = ctx.enter_context(tc.tile_pool(name="atn", bufs=1))
        ps = ctx.enter_context(tc.tile_pool(name="ps", bufs=1, space="PSUM"))

        # ---- P1a: k columns 0..511 (heads 0-7), trigger K1 gather
        def k_block(ct):
            k_ps = ps.tile([128, R], F32, tag="mm", bufs=2, name=f"k_{ct}")
            for kt in range(8):
                nc.tensor.matmul(k_ps, wkT_s[:, kt, ct * 128:(ct + 1) * 128],
                                 xT_s[:, kt, :], start=(kt == 0), stop=(kt == 7))
            nc.vector.tensor_copy(kT_ls[:, ct, :], k_ps)

        def v_block(vc, rt):
            v_ps = ps.tile([128, R], F32, tag="mm", bufs=2, name=f"v_{vc}_{rt}")
            for kt in range(8):
                nc.tensor.matmul(v_ps, xT_s[:, kt, rt * 128:(rt + 1) * 128],
                                 wvT_s[:, kt, vc * 512:(vc + 1) * 512],
                                 start=(kt == 0), stop=(kt == 7))
            dst = v_ls[:, rt, vc * 520:(vc + 1) * 520]
            nc.vector.tensor_copy(_ap(dst, [[65, 8], [1, 64]]),
                                  v_ps[:, :].rearrange("p (h e) -> p h e", e=64))

        for ct in range(4):
            k_block(ct)
        nc.gpsimd.dma_start(
            out=kA_b[:, :].rearrange("(ct p) r -> p ct r", p=128),
            in_=kT_ls[:, 0:4, :])
        nc.gpsimd.collective_compute(
            "AllGather", mybir.AluOpType.bypass,
            ins=[kA_b.opt()], outs=[kA_g.opt()], replica_groups=GROUPS)

        # ---- P2a: v columns 0..511 (heads 0-7), trigger V1 gather
        for rt in range(4):
            v_block(0, rt)
        nc.gpsimd.dma_start(
            out=vA_b[:, :].rearrange("(rt p) c -> p rt c", p=128),
            in_=v_ls[:, :, 0:520])
        nc.gpsimd.collective_compute(
            "AllGather", mybir.AluOpType.bypass,
            ins=[vA_b.opt()], outs=[vA_g.opt()], replica_groups=GROUPS)

        # ---- P1b: k columns 512..1023 (heads 8-15), trigger K2
        for ct in range(4, 8):
            k_block(ct)
        nc.gpsimd.dma_start(
            out=kB_b[:, :].rearrange("(ct p) r -> p ct r", p=128),
            in_=kT_ls[:, 4:8, :])
        nc.gpsimd.collective_compute(
            "AllGather", mybir.AluOpType.bypass,
            ins=[kB_b.opt()], outs=[kB_g.opt()], replica_groups=GROUPS)

        # ---- P2b: v columns 512..1023 (heads 8-15), trigger V2
        for rt in range(4):
            v_block(1, rt)
        nc.gpsimd.dma_start(
            out=vB_b[:, :].rearrange("(rt p) c -> p rt c", p=128),
            in_=v_ls[:, :, 520:1040])
        nc.gpsimd.collective_compute(
            "AllGather", mybir.AluOpType.bypass,
            ins=[vB_b.opt()], outs=[vB_g.opt()], replica_groups=GROUPS)

        # ---- P3: q
        for ct in range(8):
            q_ps = ps.tile([128, R], F32, tag="mm", bufs=2, name=f"q_{ct}")
            for kt in range(8):
                nc.tensor.matmul(q_ps, wqT_s[:, kt, ct * 128:(ct + 1) * 128],
                                 xT_s[:, kt, :], start=(kt == 0), stop=(kt == 7))
            nc.vector.tensor_copy(qT_s[:, ct, :], q_ps)

        # gathered V -> SBUF (on gpsimd queue, behind the collectives)
        nc.gpsimd.dma_start(out=vA_s,
                            in_=vA_g[:, :].rearrange("(kt p) c -> p kt c", p=128))
        nc.gpsimd.dma_start(out=vB_s,
                            in_=vB_g[:, :].rearrange("(kt p) c -> p kt c", p=128))

        # ---- P4: attention, per head pair
        for kp in range(8):
            kg = (kA_g if kp < 4 else kB_g)[:, :]
            kpo = kp % 4
            vs_ = vA_s if kp < 4 else vB_s
            kT_p = atn.tile([128, 4, R], BF, tag="ktp", bufs=2, name=f"ktp_{kp}")
            nc.sync.dma_start(
                out=kT_p,
                in_=bass.AP(tensor=kg.tensor,
                            offset=kg.offset + kpo * 128 * R,
                            ap=[[R, 128], [4 * 128 * R, 4], [1, R]]))
            ao = [ps.tile([65, R], F32, tag=f"ao{j}", bufs=1, name=f"ao_{kp}_{j}")
                  for j in range(2)]
            for kt in range(KT):
                sp = ps.tile([128, 2, R], F32, tag="sp", bufs=2,
                             name=f"sp_{kp}_{kt}")
                for j in range(2):
                    nc.tensor.matmul(
                        sp[:, j, :],
                        kT_p[j * 64:(j + 1) * 64, kt // 4, (kt % 4) * 128:(kt % 4) * 128 + 128],
                        qT_s[j * 64:(j + 1) * 64, kp, :],
                        start=True, stop=True)
                ex = atn.tile([128, 2, R], BF, tag="exps", bufs=6,
                              name=f"ex_{kp}_{kt}")
                nc.scalar.activation(ex, sp, mybir.ActivationFunctionType.Exp,
                                     scale=0.125)
                for j in range(2):
                    hj = 2 * kpo + j
                    nc.tensor.matmul(ao[j], vs_[:, kt, hj * 65:(hj + 1) * 65],
                                     ex[:, j, :],
                                     start=(kt == 0), stop=(kt == KT - 1))
            # drain denominators + unnormalized numerators
            for j in range(2):
                dstg = atn.tile([65, R], F32, tag="dstg", bufs=2,
                                name=f"dstg_{kp}_{j}")
                nc.vector.tensor_copy(dstg[64:65, :], ao[j][64:65, :])
                nc.gpsimd.dma_start(out=den_d[2 * kp + j:2 * kp + j + 1, :],
                                    in_=dstg[64:65, :])
                if j == 0:
                    nc.vector.tensor_copy(att_un[0:64, kp, :], ao[j][0:64, :])
                else:
                    tmp = atn.tile([64, R], F32, tag="tmpj", bufs=2,
                                   name=f"tmpj_{kp}")
                    nc.vector.tensor_copy(tmp, ao[j][0:64, :])
                    nc.gpsimd.dma_start(out=att_un[64:128, kp, :], in_=tmp)

        # ---- batched softmax denominators: one reciprocal for all 16 heads
        nc.sync.dma_start(out=den_l, in_=den_d[:, :])
        with nc.allow_low_precision(reason="softmax denom reciprocal to bf16"):
            nc.vector.reciprocal(den_r, den_l)
        nc.gpsimd.dma_start(out=den_rd, in_=den_r)
        dr = den_rd[:, :]
        for kp in range(8):
            for j in range(2):
                nc.sync.dma_start(
                    out=rb_s[j * 64:(j + 1) * 64, kp, :],
                    in_=bass.AP(tensor=dr.tensor,
                                offset=dr.offset + (2 * kp + j) * R,
                                ap=[[0, 64], [1, R]]))
        for kp in range(8):
            nc.vector.tensor_mul(att_s[:, kp, :], att_un[:, kp, :], rb_s[:, kp, :])

        # ---- P5: output projection + bias
        for ct in range(8):
            f_ps = ps.tile([128, R], F32, tag="mm", bufs=2, name=f"f_{ct}")
            for kp in range(8):
                nc.tensor.matmul(f_ps, projT_s[:, kp, ct * 128:(ct + 1) * 128],
                                 att_s[:, kp, :], start=(kp == 0), stop=False)
            nc.tensor.matmul(f_ps, biasT_s[:, ct * 128:(ct + 1) * 128], ones_s,
                             start=False, stop=True)
            f_s = atn.tile([128, R], F32, tag="fs", bufs=2, name=f"fs_{ct}")
            nc.vector.tensor_copy(f_s, f_ps)
            nc.gpsimd.dma_start(out=outT[ct * 128:(ct + 1) * 128, :], in_=f_s)

        # consume the warm-up gather so its DMA completes inside the NEFF
        warm_back = cst.tile([4, 128], BF)
        nc.sync.dma_start(out=warm_back, in_=warm_out[:, :])
    _split_multi_waits(nc)
    return nc


def _split_multi_waits(nc):
    """This container's walrus supports one sync-wait per instruction; move
    extra waits onto preceding same-engine NoOps."""
    n_new = 0
    for bb in nc.m.functions[0].blocks:
        new = []
        for ins in bb.instructions:
            si = getattr(ins, "sync_info", None)
            ow = list(si.on_wait) if si is not None and si.on_wait else []
            if len(ow) > 1:
                for w in ow[:-1]:
                    n_new += 1
                    nop = mybir.InstNoOp(
                        name=f"{ins.name}_sw{n_new}",
                        engine=ins.engine,
                        sync_info=mybir.SyncInfo(on_wait=[w], on_update=[]),
                    )
                    new.append(nop)
                ins.sync_info = mybir.SyncInfo(
                    on_wait=[ow[-1]],
                    on_update=list(si.on_update) if si.on_update else [],
                )
            new.append(ins)
        bb.instructions = new


_NC = None
_LAST = None


def _ensure_ntff_hook():
    """The agent image's antenv lacks axon_hooks; shim it and register the
    ctypes NTFF profiler from trn_boot so trace=True yields exec_time_ns."""
    import sys
    import types
    try:
        import antenv.axon_hooks  # noqa: F401
        return
    except ImportError:
        pass
    mod = types.ModuleType("antenv.axon_hooks")
    holder = [None]
    mod.set_axon_ntff_profile_hook = lambda h: holder.__setitem__(0, h)
    mod.get_axon_ntff_profile_hook = lambda: holder[0]
    sys.modules["antenv.axon_hooks"] = mod
    import antenv
    antenv.axon_hooks = mod
    try:
        sys.path.insert(0, "/root/.axon_site")
        from trn_agent_boot.trn_boot import _ntff_profile_via_ctypes
        mod.set_axon_ntff_profile_hook(
            _ntff_profile_via_ctypes("/opt/axon/libaxon_pjrt.so"))
    except Exception:
        pass


def kernel(**inputs):
    global _NC, _LAST
    bf = ml_dtypes.bfloat16
    x = np.asarray(inputs["x"], np.float32)
    qkv_w = np.asarray(inputs["qkv_w"], np.float32)
    proj_w = np.asarray(inputs["proj_w"], np.float32)
    proj_b = np.asarray(inputs["proj_b"], np.float32)
    a1 = np.asarray(inputs["lora_w1_l1"], np.float32)
    b1 = np.asarray(inputs["lora_w1_l2"], np.float32)
    a2 = np.asarray(inputs["lora_w2_l1"], np.float32)
    b2 = np.asarray(inputs["lora_w2_l2"], np.float32)

    w_eff = qkv_w + 2.0 * (b1 @ a1)
    p_eff = proj_w + 2.0 * (b2 @ a2)
    shared = {
        "wqT": np.ascontiguousarray(w_eff[0:C].T).astype(bf),
        "wkT": np.ascontiguousarray(w_eff[C:2 * C].T).astype(bf),
        "wvT": np.ascontiguousarray(w_eff[2 * C:3 * C].T).astype(bf),
        "projT": np.ascontiguousarray(p_eff.T).astype(bf),
        "biasT": np.ascontiguousarray(proj_b[None, :]).astype(bf),
    }
    in_maps = []
    for c in range(8):
        g, r = divmod(c, 4)
        m = dict(shared)
        m["xT"] = np.ascontiguousarray(x[g, r * R:(r + 1) * R, :].T).astype(bf)
        in_maps.append(m)

    if _NC is None:
        _NC = build()
    trace = os.environ.get("ATT_TRACE", "0") == "1"
    if trace:
        _ensure_ntff_hook()
    _LAST = run_bass_kernel_spmd(_NC, in_maps, core_ids=list(range(8)),
                                 trace=trace)
    out = np.empty((B, N, C), np.float32)
    for c in range(8):
        g, r = divmod(c, 4)
        out[g, r * R:(r + 1) * R, :] = np.asarray(
            _LAST.results[c]["outT"], np.float32).T
    return out


# revision 5
# speedup vs baseline: 1.4542x; 1.1354x over previous
"""Trainium2 Bass kernel: 16-head attention with LoRA (B=2, N=2048, C=1024).

Sharding: batch x sequence rows across 8 cores (core c: batch c//4, rows
(c%4)*512). Heads stay whole per core; K/V are all-gathered over the 4-core
batch group in chunks interleaved with compute. LoRA is folded into the
weights on the host (W_eff = W + 2*B@A), softmax normalization is deferred
and batched. Everything on device is computed transposed (feature dim on
partitions); the host transposes the per-core [1024, 512] output slabs back.
"""

import os
from contextlib import ExitStack

import numpy as np
import ml_dtypes

import concourse.bass as bass
import concourse.mybir as mybir
import concourse.tile as tile
from concourse.bass_utils import run_bass_kernel_spmd

B, N, C, H, D = 2, 2048, 1024, 16, 64
R = 512          # query rows per core
KT = N // 128    # 16 seq tiles of 128
BF = mybir.dt.bfloat16
F32 = mybir.dt.float32
GROUPS = [[0, 1, 2, 3], [4, 5, 6, 7]]


def _ap(src, dims):
    """Rebuild an AP keeping its partition dim but with custom free dims."""
    return bass.AP(tensor=src.tensor, offset=src.offset,
                   ap=[list(src.ap[0])] + [list(d) for d in dims])


def build():
    nc = bass.Bass()
    xT = nc.declare_dram_parameter("xT", [C, R], BF, isOutput=False)
    wkT = nc.declare_dram_parameter("wkT", [C, C], BF, isOutput=False)
    wqT = nc.declare_dram_parameter("wqT", [C, C], BF, isOutput=False)
    wvT = nc.declare_dram_parameter("wvT", [C, C], BF, isOutput=False)
    projT = nc.declare_dram_parameter("projT", [C, C], BF, isOutput=False)
    biasT = nc.declare_dram_parameter("biasT", [1, C], BF, isOutput=False)
    outT = nc.declare_dram_parameter("outT", [C, R], F32, isOutput=True)

    with tile.TileContext(nc) as tc, ExitStack() as ctx:
        dram = ctx.enter_context(tc.tile_pool(name="dram", bufs=1, space="DRAM"))
        warm_in = dram.tile([1, 128], BF)
        warm_out = dram.tile([4, 128], BF)
        kA_b = dram.tile([4 * 128, R], BF)
        kB_b = dram.tile([4 * 128, R], BF)
        kA_g = dram.tile([4 * 4 * 128, R], BF)
        kB_g = dram.tile([4 * 4 * 128, R], BF)
        vA_b = dram.tile([R, 520], BF)
        vB_b = dram.tile([R, 520], BF)
        vA_g = dram.tile([N, 520], BF)
        vB_g = dram.tile([N, 520], BF)
        den_d = dram.tile([16, R], F32)
        den_rd = dram.tile([16, R], BF)

        cst = ctx.enter_context(tc.tile_pool(name="cst", bufs=1))
        xT_s = cst.tile([128, 8, R], BF)
        nc.sync.dma_start(out=xT_s, in_=xT[:, :].rearrange("(kt p) r -> p kt r", p=128))
        wkT_s = cst.tile([128, 8, C], BF)
        nc.sync.dma_start(out=wkT_s, in_=wkT[:, :].rearrange("(kt p) c -> p kt c", p=128))
        wvT_s = cst.tile([128, 8, C], BF)
        nc.sync.dma_start(out=wvT_s, in_=wvT[:, :].rearrange("(kt p) c -> p kt c", p=128))
        wqT_s = cst.tile([128, 8, C], BF)
        nc.sync.dma_start(out=wqT_s, in_=wqT[:, :].rearrange("(kt p) c -> p kt c", p=128))
        projT_s = cst.tile([128, 8, C], BF)
        nc.sync.dma_start(out=projT_s, in_=projT[:, :].rearrange("(kt p) c -> p kt c", p=128))
        biasT_s = cst.tile([1, C], BF)
        nc.sync.dma_start(out=biasT_s, in_=biasT[:, :])

        ones_s = cst.tile([1, R], BF)
        nc.vector.memset(ones_s, 1.0)
        kT_ls = cst.tile([128, 8, R], BF)
        qT_s = cst.tile([128, 8, R], BF)
        v_ls = cst.tile([128, 4, 1040], BF)
        nc.vector.memset(v_ls, 1.0)
        vA_s = cst.tile([128, KT, 520], BF)
        vB_s = cst.tile([128, KT, 520], BF)
        att_un = cst.tile([128, 8, R], F32)
        att_s = cst.tile([128, 8, R], BF)
        rb_s = cst.tile([128, 8, R], BF)
        den_l = cst.tile([16, R], F32)
        den_r = cst.tile([16, R], BF)

        # warm-up collective at t~0: absorbs the ncfw barrier/setup latency
        warm_s = cst.tile([1, 128], BF)
        nc.vector.memset(warm_s, 1.0)
        nc.gpsimd.dma_start(out=warm_in, in_=warm_s)
        nc.gpsimd.collective_compute(
            "AllGather", mybir.AluOpType.bypass,
            ins=[warm_in.opt()], outs=[warm_out.opt()],
            replica_groups=GROUPS)

        atn = ctx.enter_context(tc.tile_pool(name="atn", bufs=1))
        ps = ctx.enter_context(tc.tile_pool(name="ps", bufs=1, space="PSUM"))

        # ---- P1a: k columns 0..511 (heads 0-7), trigger K1 gather
        def k_block(ct):
            k_ps = ps.tile([128, R], F32, tag="mm", bufs=2, name=f"k_{ct}")
            for kt in range(8):
                nc.tensor.matmul(k_ps, wkT_s[:, kt, ct * 128:(ct + 1) * 128],
                                 xT_s[:, kt, :], start=(kt == 0), stop=(kt == 7))
            nc.vector.tensor_copy(kT_ls[:, ct, :], k_ps)

        def v_block(vc, rt):
            v_ps = ps.tile([128, R], F32, tag="mm", bufs=2, name=f"v_{vc}_{rt}")
            for kt in range(8):
                nc.tensor.matmul(v_ps, xT_s[:, kt, rt * 128:(rt + 1) * 128],
                                 wvT_s[:, kt, vc * 512:(vc + 1) * 512],
                                 start=(kt == 0), stop=(kt == 7))
            dst = v_ls[:, rt, vc * 520:(vc + 1) * 520]
            nc.vector.tensor_copy(_ap(dst, [[65, 8], [1, 64]]),
                                  v_ps[:, :].rearrange("p (h e) -> p h e", e=64))

        for ct in range(4):
            k_block(ct)
        nc.gpsimd.dma_start(
            out=kA_b[:, :].rearrange("(ct p) r -> p ct r", p=128),
            in_=kT_ls[:, 0:4, :])
        nc.gpsimd.collective_compute(
            "AllGather", mybir.AluOpType.bypass,
            ins=[kA_b.opt()], outs=[kA_g.opt()], replica_groups=GROUPS)

        # ---- P2a: v columns 0..511 (heads 0-7), trigger V1 gather
        for rt in range(4):
            v_block(0, rt)
        nc.gpsimd.dma_start(
            out=vA_b[:, :].rearrange("(rt p) c -> p rt c", p=128),
            in_=v_ls[:, :, 0:520])
        nc.gpsimd.collective_compute(
            "AllGather", mybir.AluOpType.bypass,
            ins=[vA_b.opt()], outs=[vA_g.opt()], replica_groups=GROUPS)

        # ---- P1b: k columns 512..1023 (heads 8-15), trigger K2
        for ct in range(4, 8):
            k_block(ct)
        nc.gpsimd.dma_start(
            out=kB_b[:, :].rearrange("(ct p) r -> p ct r", p=128),
            in_=kT_ls[:, 4:8, :])
        nc.gpsimd.collective_compute(
            "AllGather", mybir.AluOpType.bypass,
            ins=[kB_b.opt()], outs=[kB_g.opt()], replica_groups=GROUPS)

        # ---- P2b: v columns 512..1023 (heads 8-15), trigger V2
        for rt in range(4):
            v_block(1, rt)
        nc.gpsimd.dma_start(
            out=vB_b[:, :].rearrange("(rt p) c -> p rt c", p=128),
            in_=v_ls[:, :, 520:1040])
        nc.gpsimd.collective_compute(
            "AllGather", mybir.AluOpType.bypass,
            ins=[vB_b.opt()], outs=[vB_g.opt()], replica_groups=GROUPS)

        # ---- P3: q
        for ct in range(8):
            q_ps = ps.tile([128, R], F32, tag="mm", bufs=2, name=f"q_{ct}")
            for kt in range(8):
                nc.tensor.matmul(q_ps, wqT_s[:, kt, ct * 128:(ct + 1) * 128],
                                 xT_s[:, kt, :], start=(kt == 0), stop=(kt == 7))
            nc.vector.tensor_copy(qT_s[:, ct, :], q_ps)

        # gathered V -> SBUF (on gpsimd queue, behind the collectives)
        nc.gpsimd.dma_start(out=vA_s,
                            in_=vA_g[:, :].rearrange("(kt p) c -> p kt c", p=128))
        nc.gpsimd.dma_start(out=vB_s,
                            in_=vB_g[:, :].rearrange("(kt p) c -> p kt c", p=128))

        # ---- P4: attention, per head pair
        for kp in range(8):
            kg = (kA_g if kp < 4 else kB_g)[:, :]
            kpo = kp % 4
            vs_ = vA_s if kp < 4 else vB_s
            kT_p = atn.tile([128, 4, R], BF, tag="ktp", bufs=2, name=f"ktp_{kp}")
            nc.sync.dma_start(
                out=kT_p,
                in_=bass.AP(tensor=kg.tensor,
                            offset=kg.offset + kpo * 128 * R,
                            ap=[[R, 128], [4 * 128 * R, 4], [1, R]]))
            ao = [ps.tile([65, R], F32, tag=f"ao{j}", bufs=1, name=f"ao_{kp}_{j}")
                  for j in range(2)]
            # software pipeline: attn@V for tile kt-1 is emitted after the
            # scores+exp for kt, so the PE works while ScalarE runs exp
            def av(kt, ex):
                for j in range(2):
                    hj = 2 * kpo + j
                    nc.tensor.matmul(ao[j], vs_[:, kt, hj * 65:(hj + 1) * 65],
                                     ex[:, j, :],
                                     start=(kt == 0), stop=(kt == KT - 1))
            prev_ex = None
            for kt in range(KT):
                sp = ps.tile([128, 2, R], F32, tag="sp", bufs=2,
                             name=f"sp_{kp}_{kt}")
                for j in range(2):
                    nc.tensor.matmul(
                        sp[:, j, :],
                        kT_p[j * 64:(j + 1) * 64, kt // 4, (kt % 4) * 128:(kt % 4) * 128 + 128],
                        qT_s[j * 64:(j + 1) * 64, kp, :],
                        start=True, stop=True)
                ex = atn.tile([128, 2, R], BF, tag="exps", bufs=6,
                              name=f"ex_{kp}_{kt}")
                nc.scalar.activation(ex, sp, mybir.ActivationFunctionType.Exp,
                                     scale=0.125)
                if kt > 0:
                    av(kt - 1, prev_ex)
                prev_ex = ex
            av(KT - 1, prev_ex)
            # drain denominators + unnormalized numerators
            for j in range(2):
                dstg = atn.tile([65, R], F32, tag="dstg", bufs=2,
                                name=f"dstg_{kp}_{j}")
                nc.vector.tensor_copy(dstg[64:65, :], ao[j][64:65, :])
                nc.gpsimd.dma_start(out=den_d[2 * kp + j:2 * kp + j + 1, :],
                                    in_=dstg[64:65, :])
                if j == 0:
                    nc.vector.tensor_copy(att_un[0:64, kp, :], ao[j][0:64, :])
                else:
                    tmp = atn.tile([64, R], F32, tag="tmpj", bufs=2,
                                   name=f"tmpj_{kp}")
                    nc.vector.tensor_copy(tmp, ao[j][0:64, :])
                    nc.gpsimd.dma_start(out=att_un[64:128, kp, :], in_=tmp)

        # ---- batched softmax denominators: one reciprocal for all 16 heads
        nc.sync.dma_start(out=den_l, in_=den_d[:, :])
        with nc.allow_low_precision(reason="softmax denom reciprocal to bf16"):
            nc.vector.reciprocal(den_r, den_l)
        nc.gpsimd.dma_start(out=den_rd, in_=den_r)
        dr = den_rd[:, :]
        for kp in range(8):
            for j in range(2):
                nc.sync.dma_start(
                    out=rb_s[j * 64:(j + 1) * 64, kp, :],
                    in_=bass.AP(tensor=dr.tensor,
                                offset=dr.offset + (2 * kp + j) * R,
                                ap=[[0, 64], [1, R]]))
        for kp in range(8):
            nc.vector.tensor_mul(att_s[:, kp, :], att_un[:, kp, :], rb_s[:, kp, :])

        # ---- P5: output projection + bias
        for ct in range(8):
            f_ps = ps.tile([128, R], F32, tag="mm", bufs=2, name=f"f_{ct}")
            for kp in range(8):
                nc.tensor.matmul(f_ps, projT_s[:, kp, ct * 128:(ct + 1) * 128],
                                 att_s[:, kp, :], start=(kp == 0), stop=False)
            nc.tensor.matmul(f_ps, biasT_s[:, ct * 128:(ct + 1) * 128], ones_s,
                             start=False, stop=True)
            f_s = atn.tile([128, R], F32, tag="fs", bufs=2, name=f"fs_{ct}")
            nc.vector.tensor_copy(f_s, f_ps)
            nc.gpsimd.dma_start(out=outT[ct * 128:(ct + 1) * 128, :], in_=f_s)

        # consume the warm-up gather so its DMA completes inside the NEFF
        warm_back = cst.tile([4, 128], BF)
        nc.sync.dma_start(out=warm_back, in_=warm_out[:, :])
    _split_multi_waits(nc)
    return nc


def _split_multi_waits(nc):
    """This container's walrus supports one sync-wait per instruction; move
    extra waits onto preceding same-engine NoOps."""
    n_new = 0
    for bb in nc.m.functions[0].blocks:
        new = []
        for ins in bb.instructions:
            si = getattr(ins, "sync_info", None)
            ow = list(si.on_wait) if si is not None and si.on_wait else []
            if len(ow) > 1:
                for w in ow[:-1]:
                    n_new += 1
                    nop = mybir.InstNoOp(
                        name=f"{ins.name}_sw{n_new}",
                        engine=ins.engine,
                        sync_info=mybir.SyncInfo(on_wait=[w], on_update=[]),
                    )
                    new.append(nop)
                ins.sync_info = mybir.SyncInfo(
                    on_wait=[ow[-1]],
                    on_update=list(si.on_update) if si.on_update else [],
                )
            new.append(ins)
        bb.instructions = new


_NC = None
_LAST = None


def _ensure_ntff_hook():
    """The agent image's antenv lacks axon_hooks; shim it and register the
    ctypes NTFF profiler from trn_boot so trace=True yields exec_time_ns."""
    import sys
    import types
    try:
        import antenv.axon_hooks  # noqa: F401
        return
    except ImportError:
        pass
    mod = types.ModuleType("antenv.axon_hooks")
    holder = [None]
    mod.set_axon_ntff_profile_hook = lambda h: holder.__setitem__(0, h)
    mod.get_axon_ntff_profile_hook = lambda: holder[0]
    sys.modules["antenv.axon_hooks"] = mod
    import antenv
    antenv.axon_hooks = mod
    try:
        sys.path.insert(0, "/root/.axon_site")
        from trn_agent_boot.trn_boot import _ntff_profile_via_ctypes
        mod.set_axon_ntff_profile_hook(
            _ntff_profile_via_ctypes("/opt/axon/libaxon_pjrt.so"))
    except Exception:
        pass


def kernel(**inputs):
    global _NC, _LAST
    bf = ml_dtypes.bfloat16
    x = np.asarray(inputs["x"], np.float32)
    qkv_w = np.asarray(inputs["qkv_w"], np.float32)
    proj_w = np.asarray(inputs["proj_w"], np.float32)
    proj_b = np.asarray(inputs["proj_b"], np.float32)
    a1 = np.asarray(inputs["lora_w1_l1"], np.float32)
    b1 = np.asarray(inputs["lora_w1_l2"], np.float32)
    a2 = np.asarray(inputs["lora_w2_l1"], np.float32)
    b2 = np.asarray(inputs["lora_w2_l2"], np.float32)

    w_eff = qkv_w + 2.0 * (b1 @ a1)
    p_eff = proj_w + 2.0 * (b2 @ a2)
    shared = {
        "wqT": np.ascontiguousarray(w_eff[0:C].T).astype(bf),
        "wkT": np.ascontiguousarray(w_eff[C:2 * C].T).astype(bf),
        "wvT": np.ascontiguousarray(w_eff[2 * C:3 * C].T).astype(bf),
        "projT": np.ascontiguousarray(p_eff.T).astype(bf),
        "biasT": np.ascontiguousarray(proj_b[None, :]).astype(bf),
    }
    in_maps = []
    for c in range(8):
        g, r = divmod(c, 4)
        m = dict(shared)
        m["xT"] = np.ascontiguousarray(x[g, r * R:(r + 1) * R, :].T).astype(bf)
        in_maps.append(m)

    if _NC is None:
        _NC = build()
    trace = os.environ.get("ATT_TRACE", "0") == "1"
    if trace:
        _ensure_ntff_hook()
    _LAST = run_bass_kernel_spmd(_NC, in_maps, core_ids=list(range(8)),
                                 trace=trace)
    out = np.empty((B, N, C), np.float32)
    for c in range(8):
        g, r = divmod(c, 4)
        out[g, r * R:(r + 1) * R, :] = np.asarray(
            _LAST.results[c]["outT"], np.float32).T
    return out
